# revision 18
# baseline (speedup 1.0000x reference)
"""Trainium2 Bass kernel for nn_F2FBlock (2-layer SAGEConv GNN block).

Full inputs in, full output out. Nodes sharded 6250/core across 8 NeuronCores
(padded to 6272 = 49*128). Edges sharded by dst ownership, grouped by dst tile
and (lo/hi) source-table half, chunked into 128-edge columns. Per block of G
dst tiles the kernel issues two large dma_gather instructions (one per table
half, int16 row indices) pulling pre-transformed bf16 rows (h @ w_l.T), builds
all one-hot scatter matrices for the block in two batched DVE ops (inv-degree
folded into the one-hot), and segment-sums on the TensorEngine via PSUM
accumulation; the root-path matmul accumulates into the same PSUM tile.
LayerNorm/gelu run batched over the whole 6272-node slab. The two conv layers
exchange node features with an on-device AllGather of the bf16 table.

reference math:
    shortcut = x @ sc_w.T + sc_b
    h = gelu(x @ dp_w.T + dp_b)
    h = mean_agg(h)@g1_lw.T + g1_lb + h@g1_rw.T          (SAGEConv 1)
    h = gelu(LN(h, n1_g, n1_b))
    h = mean_agg(h)@g2_lw.T + g2_lb + h@g2_rw.T          (SAGEConv 2)
    h = LN(h, n2_g, n2_b)
    out = gelu(h + shortcut)
where mean_agg(h)[i] = mean over {h[src] : (src,dst=i) in edges}.
Linearity lets us aggregate hl = h @ w_l.T and scale by 1/deg after (the
1/deg factor lives in the one-hot matrices).
"""

import numpy as np
import ml_dtypes

import concourse.bass as bass
import concourse.bacc as bacc
import concourse.tile as tile
import concourse.mybir as mybir
from concourse.masks import make_identity
from concourse.library_config import mlp as _mlp_lib

P = 128
D = 128
N = 50000
NCORE = 8
OWN = N // NCORE            # 6250 owned nodes per core
NT = (OWN + P - 1) // P     # 49 dst tiles per core
SLAB = NT * P               # 6272 padded rows per core
NPAD = SLAB * NCORE         # 50176 rows in gathered tables
HALF = 32768                # lo rows [0, HALF), hi rows [HALF, NPAD)
G = 4                       # dst tiles per gather block
EPS = 1e-5
BF = ml_dtypes.bfloat16

F32 = mybir.dt.float32
BF16 = mybir.dt.bfloat16
I16 = mybir.dt.int16
AF = mybir.ActivationFunctionType
ALU = mybir.AluOpType
AX = mybir.AxisListType


class _Layout:
    """Static (core-independent) chunk/block layout derived from the edges."""

    def __init__(self, cptA, cptB):
        self.cptA = tuple(int(v) for v in cptA)
        self.cptB = tuple(int(v) for v in cptB)
        self.blocks = [list(range(b, min(NT, b + G))) for b in range(0, NT, G)]
        # global chunk-column layout: per block, all lo cols then all hi cols
        self.col0 = []          # first global col of each block
        self.ncols = []         # cols per block
        self.nlo = []           # lo cols per block
        self.tile_lo = {}       # t -> (start col, count) local-to-block? global
        self.tile_hi = {}
        self.idx16_lo = []      # idx_s col offset of each block's lo segment
        self.idx16_hi = []
        col = 0
        i16 = 0
        for tiles in self.blocks:
            self.col0.append(col)
            nlo_b = sum(self.cptA[t] for t in tiles)
            nhi_b = sum(self.cptB[t] for t in tiles)
            self.nlo.append(nlo_b)
            self.ncols.append(nlo_b + nhi_b)
            c = col
            for t in tiles:
                self.tile_lo[t] = (c, self.cptA[t])
                c += self.cptA[t]
            for t in tiles:
                self.tile_hi[t] = (c, self.cptB[t])
                c += self.cptB[t]
            col = c
            self.idx16_lo.append(i16)
            i16 += nlo_b * P // 16
            self.idx16_hi.append(i16)
            i16 += nhi_b * P // 16
        self.nch_tot = col
        self.nidx16 = i16
        self.cblk = max(self.ncols)

    def key(self):
        return (self.cptA, self.cptB)


def _build_nc(layout: _Layout, reps: int = 1, sim: bool = False,
              stub_cc: bool = False, stage: int = 5):
    nc = bacc.Bacc("TRN2", target_bir_lowering=False, debug=False,
                   num_devices=1 if sim else NCORE)
    gelu_f = AF.Tanh if sim else AF.Gelu  # CoreSim has no Gelu table
    stub_cc = stub_cc or sim

    # ---- I/O ----
    x_t = nc.dram_tensor("x_t", [P, SLAB], BF16, kind="ExternalInput")
    idx_in = nc.dram_tensor("idx_in", [P, layout.nidx16], I16,
                            kind="ExternalInput")
    w_all = nc.dram_tensor("w_all", [P, layout.nch_tot * P], BF16,
                           kind="ExternalInput")
    w_names = ["w_dp", "w_sc", "w_g1l", "w_g1r", "w_g2l", "w_g2r"]
    w_in = {n: nc.dram_tensor(n, [D, D], BF16, kind="ExternalInput")
            for n in w_names}
    dp_b = nc.dram_tensor("dp_b", [D, 1], F32, kind="ExternalInput")
    r_names = ["sc_b", "g1_lb", "g2_lb", "n1_g", "n1_b", "n2_g", "n2_b"]
    r_in = {n: nc.dram_tensor(n, [P, D], F32, kind="ExternalInput")
            for n in r_names}
    out = nc.dram_tensor("out", [SLAB, D], F32, kind="ExternalOutput")

    NCH = layout.nch_tot
    CBLK = layout.cblk

    with tile.TileContext(nc) as tc:
        with (
            tc.tile_pool(name="const", bufs=1) as cp,
            tc.tile_pool(name="work", bufs=4) as wp,
            tc.tile_pool(name="msgs", bufs=2) as mp,
            tc.tile_pool(name="oneh", bufs=2) as op_,
            tc.tile_pool(name="psA", bufs=4, space="PSUM") as pA,
            tc.tile_pool(name="psB", bufs=4, space="PSUM") as pB,
            tc.tile_pool(name="dram", bufs=1, space="DRAM") as dp_,
        ):
            nc.gpsimd.load_library(_mlp_lib)

            # ---- constants into SBUF ----
            xt_s = cp.tile([P, SLAB], BF16, tag="xt")
            nc.sync.dma_start(out=xt_s[:], in_=x_t[:])
            idx_s = cp.tile([P, layout.nidx16], I16, tag="idx")
            nc.sync.dma_start(out=idx_s[:], in_=idx_in[:])
            w_s = {}
            for n in w_names:
                w_s[n] = cp.tile([D, D], BF16, tag=n, name=n)
                nc.sync.dma_start(out=w_s[n][:], in_=w_in[n][:])
            dpb_s = cp.tile([D, 1], F32, tag="dpb")
            nc.sync.dma_start(out=dpb_s[:], in_=dp_b[:])
            r_s = {}
            for n in r_names:
                r_s[n] = cp.tile([P, D], F32, tag=n, name=n)
                nc.sync.dma_start(out=r_s[n][:], in_=r_in[n][:])
            ident = cp.tile([P, P], F32, tag="ident")
            make_identity(nc, ident[:])

            # persistent slabs
            h0fm = [cp.tile([P, P], BF16, tag=f"h0fm{i}", name=f"h0fm{i}")
                    for i in range(NT)]
            h1fm = [cp.tile([P, P], BF16, tag=f"h1fm{i}", name=f"h1fm{i}")
                    for i in range(NT)]
            shct = cp.tile([P, SLAB], BF16, tag="shct")
            hslab = cp.tile([P, SLAB], F32, tag="hslab")
            hscr = cp.tile([P, SLAB], F32, tag="hscr")
            stat_sum = cp.tile([P, NT], F32, tag="stat_sum")
            stat_sq = cp.tile([P, NT], F32, tag="stat_sq")
            stat_mu = cp.tile([P, NT], F32, tag="stat_mu")
            stat_var = cp.tile([P, NT], F32, tag="stat_var")
            stat_sd = cp.tile([P, NT], F32, tag="stat_sd")
            stat_rstd = cp.tile([P, NT], F32, tag="stat_rstd")

            # internal DRAM state
            hl1slab = dp_.tile([SLAB, D], BF16)
            hl1full = dp_.tile([NPAD, D], BF16)
            hl2slab = dp_.tile([SLAB, D], BF16)
            hl2full = dp_.tile([NPAD, D], BF16)

            def allgather(slab, full):
                if stub_cc:
                    for r in range(NCORE):
                        nc.sync.dma_start(
                            out=full[r * SLAB:(r + 1) * SLAB, :], in_=slab[:])
                else:
                    nc.gpsimd.collective_compute(
                        "AllGather", ALU.bypass,
                        replica_groups=[list(range(NCORE))],
                        ins=[slab.opt()], outs=[full.opt()])

            def conv_layer(table, rfm_tiles, w_r, bias_name, gather_only=False):
                """Aggregation + root path for one SAGEConv; writes hslab."""
                for b, tiles in enumerate(layout.blocks):
                    nlo_b = layout.nlo[b]
                    nhi_b = layout.ncols[b] - nlo_b
                    ncb = layout.ncols[b]
                    c0 = layout.col0[b]
                    msgs = mp.tile([P, CBLK, P], BF16, tag="msgs")

                    # dma_gather is limited to 1024 descriptors (16KB SWDGE
                    # scratch ring) per instruction -> <= 8 chunk-columns.
                    def emit_gathers(base, ncols_h, i16off, tbl):
                        done = 0
                        while done < ncols_h:
                            take = min(8, ncols_h - done)
                            nc.gpsimd.dma_gather(
                                msgs[:, base + done:base + done + take, :],
                                tbl,
                                idx_s[:, i16off + done * 8:
                                      i16off + (done + take) * 8],
                                take * P, take * P, D)
                            done += take

                    if nlo_b:
                        emit_gathers(0, nlo_b, layout.idx16_lo[b],
                                     table[:HALF, :])
                    if nhi_b:
                        emit_gathers(nlo_b, nhi_b, layout.idx16_hi[b],
                                     table[HALF:, :])
                    w1 = op_.tile([P, CBLK * P], BF16, tag="oneh")
                    nc.sync.dma_start(
                        out=w1[:, :ncb * P],
                        in_=w_all[:, c0 * P:(c0 + ncb) * P])
                    if gather_only:
                        # consume msgs/w1 cheaply so pool rotation still works
                        junk = pA.tile([P, D], F32, space="PSUM", tag="agg")
                        nc.tensor.matmul(junk[:], lhsT=w1[:, :P],
                                         rhs=msgs[:, 0, :], start=True, stop=True)
                        continue
                    for t in tiles:
                        ps = pA.tile([P, D], F32, space="PSUM", tag="agg")
                        lo0, nlo_t = layout.tile_lo[t]
                        hi0, nhi_t = layout.tile_hi[t]
                        cols = ([lo0 - c0 + k for k in range(nlo_t)] +
                                [hi0 - c0 + k for k in range(nhi_t)])
                        for k, c in enumerate(cols):
                            nc.tensor.matmul(
                                ps[:], lhsT=w1[:, c * P:(c + 1) * P],
                                rhs=msgs[:, c, :], start=(k == 0), stop=False)
                        nc.tensor.matmul(ps[:], lhsT=rfm_tiles[t][:], rhs=w_r[:],
                                         start=(not cols), stop=True)
                        nc.vector.tensor_tensor(
                            out=hslab[:, t * P:(t + 1) * P], in0=ps[:],
                            in1=r_s[bias_name][:], op=ALU.add)

            def layer_norm_slab(gname, bname):
                """Batched LN over hslab (in place)."""
                h3 = hslab[:].rearrange("p (t f) -> p t f", t=NT)
                s3 = hscr[:].rearrange("p (t f) -> p t f", t=NT)
                nc.vector.tensor_reduce(out=stat_sum[:], in_=h3, axis=AX.X,
                                        op=ALU.add)
                nc.vector.tensor_tensor(out=hscr[:], in0=hslab[:], in1=hslab[:],
                                        op=ALU.mult)
                nc.vector.tensor_reduce(out=stat_sq[:], in_=s3, axis=AX.X,
                                        op=ALU.add)
                nc.vector.tensor_scalar_mul(out=stat_mu[:], in0=stat_sum[:],
                                            scalar1=1.0 / D)
                nc.vector.tensor_tensor(out=stat_var[:], in0=stat_mu[:],
                                        in1=stat_mu[:], op=ALU.mult)
                nc.vector.scalar_tensor_tensor(
                    out=stat_var[:], in0=stat_sq[:], scalar=1.0 / D,
                    in1=stat_var[:], op0=ALU.mult, op1=ALU.subtract)
                nc.vector.tensor_scalar_add(out=stat_var[:], in0=stat_var[:],
                                            scalar1=EPS)
                nc.scalar.activation(out=stat_sd[:], in_=stat_var[:],
                                     func=AF.Sqrt)
                nc.vector.reciprocal(out=stat_rstd[:], in_=stat_sd[:])
                nc.vector.tensor_tensor(
                    out=h3, in0=h3, in1=stat_mu[:].to_broadcast([P, NT, P]),
                    op=ALU.subtract)
                nc.vector.tensor_tensor(
                    out=h3, in0=h3, in1=stat_rstd[:].to_broadcast([P, NT, P]),
                    op=ALU.mult)
                g3 = r_s[gname][:].rearrange("p (o f) -> p o f", o=1)
                b3 = r_s[bname][:].rearrange("p (o f) -> p o f", o=1)
                nc.vector.tensor_tensor(out=h3, in0=h3,
                                        in1=g3.to_broadcast([P, NT, P]),
                                        op=ALU.mult)
                nc.vector.tensor_tensor(out=h3, in0=h3,
                                        in1=b3.to_broadcast([P, NT, P]),
                                        op=ALU.add)

            for _rep in range(reps):
                # ---- phase A: hl1 chain (feeds AllGather 1 asap) ----
                for i in range(NT):
                    xt_i = xt_s[:, i * P:(i + 1) * P]
                    ph = pB.tile([P, P], F32, space="PSUM", tag="pd")
                    nc.tensor.matmul(ph[:], lhsT=w_s["w_dp"][:], rhs=xt_i,
                                     start=True, stop=True)
                    nc.scalar.activation(out=h0fm[i][:], in_=ph[:],
                                         func=gelu_f, bias=dpb_s[:])
                    p2 = pB.tile([P, P], F32, space="PSUM", tag="pd")
                    nc.tensor.matmul(p2[:], lhsT=h0fm[i][:], rhs=w_s["w_g1l"][:],
                                     start=True, stop=True)
                    hl1bf = wp.tile([P, P], BF16, tag="hl1bf")
                    nc.vector.tensor_copy(out=hl1bf[:], in_=p2[:])
                    nc.sync.dma_start(out=hl1slab[i * P:(i + 1) * P, :],
                                      in_=hl1bf[:])

                allgather(hl1slab, hl1full)

                # ---- phase B: shortcut (overlaps AllGather 1) ----
                for i in range(NT):
                    xt_i = xt_s[:, i * P:(i + 1) * P]
                    p3 = pB.tile([P, P], F32, space="PSUM", tag="pd")
                    nc.tensor.matmul(p3[:], lhsT=xt_i, rhs=w_s["w_sc"][:],
                                     start=True, stop=True)
                    nc.vector.tensor_tensor(
                        out=shct[:, i * P:(i + 1) * P], in0=p3[:],
                        in1=r_s["sc_b"][:], op=ALU.add)

                if stage <= 1:
                    nc.vector.tensor_copy(out=hscr[:], in_=shct[:])
                    for t in range(NT):
                        nc.sync.dma_start(out=out[t * P:(t + 1) * P, :],
                                          in_=hscr[:, t * P:(t + 1) * P])
                    continue
                if stage == 15:  # gathers + W loads only
                    conv_layer(hl1full, h0fm, w_s["w_g1r"], "g1_lb",
                               gather_only=True)
                    nc.vector.tensor_copy(out=hscr[:], in_=shct[:])
                    for t in range(NT):
                        nc.sync.dma_start(out=out[t * P:(t + 1) * P, :],
                                          in_=hscr[:, t * P:(t + 1) * P])
                    continue

                # ---- layer 1: aggregation + root, LN, gelu ----
                conv_layer(hl1full, h0fm, w_s["w_g1r"], "g1_lb")
                if stage <= 2:
                    for t in range(NT):
                        nc.sync.dma_start(out=out[t * P:(t + 1) * P, :],
                                          in_=hslab[:, t * P:(t + 1) * P])
                    continue
                layer_norm_slab("n1_g", "n1_b")
                nc.scalar.activation(out=hscr[:], in_=hslab[:], func=gelu_f)
                if stage <= 3:
                    for t in range(NT):
                        nc.sync.dma_start(out=out[t * P:(t + 1) * P, :],
                                          in_=hscr[:, t * P:(t + 1) * P])
                    continue

                # transpose h1 + hl2 chain (feeds AllGather 2)
                for t in range(NT):
                    tp = pB.tile([P, P], F32, space="PSUM", tag="pd")
                    nc.tensor.transpose(out=tp[:],
                                        in_=hscr[:, t * P:(t + 1) * P],
                                        identity=ident[:])
                    nc.vector.tensor_copy(out=h1fm[t][:], in_=tp[:])
                    p5 = pB.tile([P, P], F32, space="PSUM", tag="pd")
                    nc.tensor.matmul(p5[:], lhsT=h1fm[t][:],
                                     rhs=w_s["w_g2l"][:], start=True, stop=True)
                    hl2bf = wp.tile([P, P], BF16, tag="hl2bf")
                    nc.vector.tensor_copy(out=hl2bf[:], in_=p5[:])
                    nc.sync.dma_start(out=hl2slab[t * P:(t + 1) * P, :],
                                      in_=hl2bf[:])

                allgather(hl2slab, hl2full)

                # ---- layer 2 + final ----
                conv_layer(hl2full, h1fm, w_s["w_g2r"], "g2_lb")
                if stage <= 4:
                    for t in range(NT):
                        nc.sync.dma_start(out=out[t * P:(t + 1) * P, :],
                                          in_=hslab[:, t * P:(t + 1) * P])
                    continue
                layer_norm_slab("n2_g", "n2_b")
                nc.vector.tensor_tensor(out=hslab[:], in0=hslab[:],
                                        in1=shct[:], op=ALU.add)
                nc.scalar.activation(out=hscr[:], in_=hslab[:], func=gelu_f)
                for t in range(NT):
                    nc.sync.dma_start(out=out[t * P:(t + 1) * P, :],
                                      in_=hscr[:, t * P:(t + 1) * P])

    nc.compile()
    return nc


# ---------------------------------------------------------------------------
# host side: preprocessing + PJRT runner
# ---------------------------------------------------------------------------

class _Runner:
    """Reusable jitted PJRT executor for a compiled Bass module (axon)."""

    def __init__(self, nc, n_cores):
        import jax
        from jax.sharding import Mesh, PartitionSpec
        from jax.experimental.shard_map import shard_map
        from concourse.bass2jax import (_bass_exec_p, install_neuronx_cc_hook,
                                        partition_id_tensor)
        self.jax = jax
        install_neuronx_cc_hook()
        self.n_cores = n_cores
        pname = nc.partition_id_tensor.name if nc.partition_id_tensor else None
        in_names, out_names, out_avals, zero_outs = [], [], [], []
        for alloc in nc.m.functions[0].allocations:
            if not isinstance(alloc, mybir.MemoryLocationSet):
                continue
            name = alloc.memorylocations[0].name
            if alloc.kind == "ExternalInput":
                if name != pname:
                    in_names.append(name)
            elif alloc.kind == "ExternalOutput":
                shape = tuple(alloc.tensor_shape)
                dtype = mybir.dt.np(alloc.dtype)
                out_names.append(name)
                out_avals.append(jax.core.ShapedArray(shape, dtype))
                zero_outs.append(np.zeros(shape, dtype))
        self.in_names, self.out_names = in_names, out_names
        self.out_avals, self.zero_outs = out_avals, zero_outs
        n_params, n_outs = len(in_names), len(out_names)
        all_in = list(in_names) + list(out_names)
        if pname is not None:
            all_in.append(pname)

        def _body(*args):
            operands = list(args)
            if pname is not None:
                operands.append(partition_id_tensor())
            outs = _bass_exec_p.bind(
                *operands, out_avals=tuple(out_avals), in_names=tuple(all_in),
                out_names=tuple(out_names), lowering_input_output_aliases=(),
                sim_require_finite=False, sim_require_nnan=False, nc=nc)
            return tuple(outs)

        devices = jax.devices()[:n_cores]
        mesh = Mesh(np.asarray(devices), ("core",))
        self.mesh = mesh
        in_specs = (PartitionSpec("core"),) * (n_params + n_outs)
        out_specs = (PartitionSpec("core"),) * n_outs
        self.fn = jax.jit(
            shard_map(_body, mesh=mesh, in_specs=in_specs,
                      out_specs=out_specs, check_rep=False),
            keep_unused=True)

    def prepare_args(self, in_maps):
        """Host->device transfer once; returns device-resident args."""
        import jax
        from jax.sharding import NamedSharding, PartitionSpec
        n = self.n_cores
        args = [np.concatenate([np.asarray(in_maps[c][nm]) for c in range(n)], 0)
                for nm in self.in_names]
        args += [np.zeros((n * z.shape[0], *z.shape[1:]), z.dtype)
                 for z in self.zero_outs]
        sh = NamedSharding(self.mesh, PartitionSpec("core"))
        return [jax.device_put(a, sh) for a in args]

    def run_prepared(self, dev_args):
        out_arrs = self.fn(*dev_args)
        for o in out_arrs:
            o.block_until_ready()
        return out_arrs

    def run(self, in_maps):
        n = self.n_cores
        out_arrs = self.run_prepared(self.prepare_args(in_maps))
        return [
            {nm: np.asarray(out_arrs[i]).reshape(n, *self.out_avals[i].shape)[c]
             for i, nm in enumerate(self.out_names)}
            for c in range(n)
        ]


_CACHE = {}


def _get_runner(layout, reps=1):
    key = (layout.key(), reps)
    if key not in _CACHE:
        nc = _build_nc(layout, reps)
        _CACHE[key] = _Runner(nc, NCORE)
    return _CACHE[key]


def _wrap16(idx):
    """edge e -> [p = e%16, col = e//16], replicated across the 8 Q7 cores."""
    n = idx.shape[0]
    w = idx.reshape(n // 16, 16).T
    return np.tile(w, (8, 1))


def _preprocess(x, edges, dp_w, dp_b, sc_w, sc_b, g1_lw, g1_lb, g1_rw, n1_g,
                n1_b, g2_lw, g2_lb, g2_rw, n2_g, n2_b):
    src = np.asarray(edges[0], dtype=np.int64)
    dst = np.asarray(edges[1], dtype=np.int64)
    x = np.asarray(x, dtype=np.float32)

    cnt = np.bincount(dst, minlength=N).astype(np.float32)
    inv = (1.0 / np.maximum(cnt, 1.0)).astype(np.float32)
    pid = (src // OWN) * SLAB + (src % OWN)
    core_of = dst // OWN
    dloc = dst % OWN
    t_of = dloc // P
    d_of = dloc % P
    ishi = (pid >= HALF).astype(np.int64)

    # per (core, tile, half) edge counts -> static chunk layout
    key = (core_of * NT + t_of) * 2 + ishi
    counts = np.bincount(key, minlength=NCORE * NT * 2).reshape(NCORE, NT, 2)
    cptA = np.ceil(counts[:, :, 0].max(axis=0) / P).astype(int)
    cptB = np.ceil(counts[:, :, 1].max(axis=0) / P).astype(int)
    layout = _Layout(cptA, cptB)

    # per-core data fill
    order = np.argsort(key, kind="stable")
    ins = []
    for c in range(NCORE):
        dl_arr = np.full((layout.nch_tot, P), -1.0, np.float32)
        inv_arr = np.zeros((layout.nch_tot, P), np.float32)
        idx_arr = np.zeros((P, layout.nidx16), np.int16)
        # boundaries of (c, t, h) groups inside `order`
        base = np.concatenate([[0], np.cumsum(counts.reshape(-1))])
        for b, tiles in enumerate(layout.blocks):
            for half, cpt_arr, i16_off, tile_map, off_base in (
                (0, layout.cptA, layout.idx16_lo[b], layout.tile_lo, HALF),
                (1, layout.cptB, layout.idx16_hi[b], layout.tile_hi, HALF),
            ):
                seg = []
                for t in tiles:
                    gi = (c * NT + t) * 2 + half
                    lo, hi = base[gi], base[gi + 1]
                    e = order[lo:hi]
                    npad = cpt_arr[t] * P
                    pid_t = np.zeros(npad, np.int64)
                    pid_t[:hi - lo] = pid[e] - (HALF if half else 0)
                    seg.append(pid_t)
                    col0, ncol = tile_map[t]
                    if ncol == 0:
                        continue
                    dv = np.full(npad, -1.0, np.float32)
                    dv[:hi - lo] = d_of[e]
                    iv = np.zeros(npad, np.float32)
                    iv[:hi - lo] = inv[dst[e]]
                    dl_arr[col0:col0 + ncol] = dv.reshape(ncol, P)
                    inv_arr[col0:col0 + ncol] = iv.reshape(ncol, P)
                if seg:
                    seg = np.concatenate(seg)
                    if seg.size:
                        w = _wrap16(seg.astype(np.int16))
                        idx_arr[:, i16_off:i16_off + seg.size // 16] = w

        x_slab = np.zeros((SLAB, D), np.float32)
        base_n = c * OWN
        x_slab[:OWN] = x[base_n:base_n + OWN]

        c_idx, p_idx = np.nonzero(dl_arr >= 0)
        d_idx = dl_arr[c_idx, p_idx].astype(np.int64)
        w_arr = np.zeros((P, layout.nch_tot * P), BF)
        w_arr[p_idx, c_idx * P + d_idx] = inv_arr[c_idx, p_idx].astype(BF)

        ins.append({
            "x_t": np.ascontiguousarray(x_slab.T).astype(BF),
            "idx_in": idx_arr,
            "w_all": w_arr,
        })

    shared = {
        "w_dp": np.ascontiguousarray(np.asarray(dp_w, np.float32).T).astype(BF),
        "w_sc": np.ascontiguousarray(np.asarray(sc_w, np.float32).T).astype(BF),
        "w_g1l": np.ascontiguousarray(np.asarray(g1_lw, np.float32).T).astype(BF),
        "w_g1r": np.ascontiguousarray(np.asarray(g1_rw, np.float32).T).astype(BF),
        "w_g2l": np.ascontiguousarray(np.asarray(g2_lw, np.float32).T).astype(BF),
        "w_g2r": np.ascontiguousarray(np.asarray(g2_rw, np.float32).T).astype(BF),
        "dp_b": np.asarray(dp_b, np.float32).reshape(D, 1),
        "sc_b": np.tile(np.asarray(sc_b, np.float32), (P, 1)),
        "g1_lb": np.tile(np.asarray(g1_lb, np.float32), (P, 1)),
        "g2_lb": np.tile(np.asarray(g2_lb, np.float32), (P, 1)),
        "n1_g": np.tile(np.asarray(n1_g, np.float32), (P, 1)),
        "n1_b": np.tile(np.asarray(n1_b, np.float32), (P, 1)),
        "n2_g": np.tile(np.asarray(n2_g, np.float32), (P, 1)),
        "n2_b": np.tile(np.asarray(n2_b, np.float32), (P, 1)),
    }
    for m in ins:
        m.update(shared)
    return ins, layout


def kernel(**inputs) -> np.ndarray:
    in_maps, layout = _preprocess(**inputs)
    runner = _get_runner(layout)
    res = runner.run(in_maps)
    return np.concatenate([res[c]["out"][:OWN] for c in range(NCORE)], axis=0)
